# revision 6
# baseline (speedup 1.0000x reference)
"""BiLSTM tagger kernel for 8 Trainium2 NeuronCores — direction-sharded.

Cores 0-3 run the forward direction for sequence groups of 32; cores 4-7
run the backward direction for the same groups. Each core therefore runs
ONE lstm scan per layer over 32 sequences, so the W_hh stream (the PE
bottleneck) is amortized over twice the batch of the old 16-seq/2-scan
layout. Between layers the f/b core pairs exchange hidden states with a
pairwise AllGather; the classifier is computed as two partial products
(W_cls split by direction) summed on the host.

Gate layout is hidden-block permuted: slice n in {0,1} holds columns
[i_n | f_n | o_n | g_n] for hidden cols [256n, 256n+256), so each block's
activation/elementwise tail only depends on its own psum slice and the
next step's k-chunk matmuls can start as soon as that block's h lands.
All gates go through one sigmoid pass (W_g and b_g are pre-scaled by 2 on
the host; tanh(g) = 2*sigmoid(2g) - 1 is recovered on VectorE).

Sequence-length masking is free: scan outputs are written unmasked and
consumers (layer-2 projection, classifier) gather rows through
host-computed indices that redirect out-of-range positions to a zeroed
pad row.
"""

import sys

for _p in ("/opt/trn_rl_repo",):
    if _p not in sys.path:
        sys.path.append(_p)

import numpy as np
import ml_dtypes

import concourse.bass as bass
import concourse.tile as tile
from concourse import bacc, mybir
from concourse.bass import IndirectOffsetOnAxis
from concourse.bass_utils import run_bass_kernel_spmd

F32 = mybir.dt.float32
BF16 = mybir.dt.bfloat16
I32 = mybir.dt.int32
AF = mybir.ActivationFunctionType
ALU = mybir.AluOpType

B, T, V, E, H, TAGS = 128, 512, 50000, 256, 512, 64
NC = 8
BL = 32            # sequences per core (one direction)
G = 4 * H          # 2048 gate cols
NBLK = 2           # hidden blocks per step
HB = H // NBLK     # 256 hidden cols per block
SL = G // NBLK     # 1024 gate cols per block-slice
KH = H // 128      # 4 k-chunks for the recurrent contraction

# permuted gate order: block n -> [i_n | f_n | o_n | g_n], each HB wide
_QGATE = [0, 1, 3, 2]  # pytorch i,f,g,o -> i,f,o,g
_GPERM = np.concatenate([
    np.arange(q * H + HB * n, q * H + HB * (n + 1))
    for n in range(NBLK) for q in _QGATE])


def _build(nc, Tn=T, Bl=BL, TC=2, RC=4):
    ntok = Bl * Tn
    nchunk = ntok // 128
    NP = ntok + 128            # rows incl. zero pad block
    KE = E // 128              # 2
    KH2 = 2 * H // 128         # 8
    assert ntok % 128 == 0

    e_rows = nc.dram_tensor("e_rows", [ntok, E], BF16, kind="ExternalInput")
    p2f_idx = nc.dram_tensor("p2f_idx", [128, nchunk], I32, kind="ExternalInput")
    p2b_idx = nc.dram_tensor("p2b_idx", [128, nchunk], I32, kind="ExternalInput")
    ident = nc.dram_tensor("ident", [Bl, Bl], BF16, kind="ExternalInput")

    wih1 = nc.dram_tensor("wihT_1", [E, G], BF16, kind="ExternalInput")
    wih2 = nc.dram_tensor("wihT_2", [2 * H, G], BF16, kind="ExternalInput")
    whh = {1: nc.dram_tensor("whhT_1", [H, G], BF16, kind="ExternalInput"),
           2: nc.dram_tensor("whhT_2", [H, G], BF16, kind="ExternalInput")}
    biasd = {1: nc.dram_tensor("bias_1", [128, G], F32, kind="ExternalInput"),
             2: nc.dram_tensor("bias_2", [128, G], F32, kind="ExternalInput")}
    wcls = nc.dram_tensor("wclsT", [H, TAGS], BF16, kind="ExternalInput")

    gx = {1: nc.dram_tensor("gx_1", [ntok, G], BF16),
          2: nc.dram_tensor("gx_2", [ntok, G], BF16)}
    h1 = nc.dram_tensor("h1", [NP, H], BF16)
    h1_all = nc.dram_tensor("h1_all", [2 * NP, H], BF16)
    h2 = nc.dram_tensor("h2", [NP, H], BF16)
    logitsT = nc.dram_tensor("logitsT", [TAGS, ntok], F32, kind="ExternalOutput")

    with tile.TileContext(nc) as tc:
        with tc.tile_pool(name="const", bufs=1) as cpool:
            def load_const(nm, shape, dt, src_ap):
                t = cpool.tile(shape, dt, name=nm, tag=nm)
                nc.gpsimd.dma_start(t[:], src_ap)
                return t

            p2f_sb = load_const("p2f_sb", [128, nchunk], I32, p2f_idx[:])
            p2b_sb = load_const("p2b_sb", [128, nchunk], I32, p2b_idx[:])
            id_sb = load_const("id_sb", [Bl, Bl], BF16, ident[:])
            bias_sb = {l: load_const(f"bias_sb_{l}", [128, G], F32, biasd[l][:])
                       for l in (1, 2)}
            wcls_sb = cpool.tile([128, KH, TAGS], BF16, name="wcls_sb")
            for k in range(KH):
                nc.gpsimd.dma_start(wcls_sb[:, k, :], wcls[128 * k:128 * (k + 1), :])

            # zero the pad row blocks of h1/h2
            zpad = cpool.tile([128, H], BF16, name="zpad", tag="zpad")
            nc.vector.memset(zpad[:], 0.0)
            nc.gpsimd.dma_start(h1[ntok:NP, :], zpad[:])
            nc.gpsimd.dma_start(h2[ntok:NP, :], zpad[:])

            # ---- layer 1: embedding gather + input projection ----
            _proj_phase(nc, tc, nchunk, KE, wih1, bias_sb[1], gx[1],
                        src=("rows", e_rows, None))
            # ---- layer 1 scan ----
            _scan_phase(nc, tc, Tn, Bl, TC, RC, whh[1], gx[1], h1, id_sb)

            # ---- exchange hidden states between f/b core pairs ----
            nc.gpsimd.collective_compute(
                "AllGather", ALU.bypass,
                replica_groups=[[0, 4], [1, 5], [2, 6], [3, 7]],
                ins=[h1[:].opt()], outs=[h1_all[:].opt()])

            # ---- layer 2 projection ----
            _proj_phase(nc, tc, nchunk, KH2, wih2, bias_sb[2], gx[2],
                        src=("pair", h1_all, (p2f_sb, p2b_sb)))
            # ---- layer 2 scan with the partial classifier fused in:
            # each ring flush covers exactly one 128-row t-major chunk of h2,
            # so the classifier chunk runs under the scan instead of as a
            # 1ms latency-bound tail phase ----
            _scan_phase(nc, tc, Tn, Bl, TC, RC, whh[2], gx[2], h2, id_sb,
                        cls=(wcls_sb, logitsT))
    return nc


def _proj_phase(nc, tc, nchunk, KD, wih, bias_sb, gxd, src):
    """gx = input @ wih + bias, token chunks of 128 in scan order."""
    D = KD * 128
    kind, dsrc, idx = src
    with tc.tile_pool(name="pw", bufs=1) as wpool, \
         tc.tile_pool(name="pg", bufs=3) as gpool, \
         tc.tile_pool(name="pgT", bufs=3) as tpool, \
         tc.tile_pool(name="pps", bufs=4, space="PSUM") as ppool, \
         tc.tile_pool(name="pout", bufs=3) as opool:
        wsb = wpool.tile([128, KD, G], BF16, name="wih_sb")
        for k in range(KD):
            nc.gpsimd.dma_start(wsb[:, k, :], wih[128 * k:128 * (k + 1), :])
        for c in range(nchunk):
            if kind == "rows":
                xin = gpool.tile([128, D], BF16, tag="e16")
                nc.gpsimd.dma_start(xin[:], dsrc[128 * c:128 * (c + 1), :])
            else:
                fidx, bidx = idx
                xin = gpool.tile([128, D], BF16, tag="e16")
                nc.gpsimd.indirect_dma_start(
                    out=xin[:, 0:D // 2], out_offset=None, in_=dsrc[:],
                    in_offset=IndirectOffsetOnAxis(ap=fidx[:, c:c + 1], axis=0))
                nc.gpsimd.indirect_dma_start(
                    out=xin[:, D // 2:D], out_offset=None, in_=dsrc[:],
                    in_offset=IndirectOffsetOnAxis(ap=bidx[:, c:c + 1], axis=0))
            xT = tpool.tile([128, KD, 128], BF16, tag="xT")
            for k in range(KD):
                nc.sync.dma_start_transpose(
                    xT[:, k, :], xin[:, 128 * k:128 * (k + 1)])
            gout = opool.tile([128, G], BF16, tag="gout")
            for n in range(G // 512):
                ps = ppool.tile([128, 512], F32, name="pps")
                for k in range(KD):
                    nc.tensor.matmul(
                        ps[:], xT[:, k, :], wsb[:, k, 512 * n:512 * (n + 1)],
                        start=(k == 0), stop=(k == KD - 1))
                nc.vector.tensor_tensor(
                    out=gout[:, 512 * n:512 * (n + 1)], in0=ps[:],
                    in1=bias_sb[:, 512 * n:512 * (n + 1)], op=ALU.add)
            nc.gpsimd.dma_start(gxd[128 * c:128 * (c + 1), :], gout[:])


def _scan_phase(nc, tc, Tn, Bl, TC, RC, whhd, gxd, hout, id_sb, cls=None):
    """One-direction scan over Bl sequences; hidden-block pipelined."""
    # token rows are TIME-MAJOR: row = t*Bl + b, so layer-2 projection
    # chunks stream in time order and overlap under the scan
    gxv = gxd.ap().rearrange("(t b) d -> b t d", b=Bl)
    houtv = hout.ap()[0:Bl * Tn, :].rearrange("(t b) d -> b t d", b=Bl)
    with tc.tile_pool(name="sw", bufs=1) as wpool, \
         tc.tile_pool(name="sgx", bufs=4) as gxpool, \
         tc.tile_pool(name="sst", bufs=1) as stpool, \
         tc.tile_pool(name="sps", bufs=1, space="PSUM") as pspool, \
         tc.tile_pool(name="spsT", bufs=2, space="PSUM") as tppool, \
         tc.tile_pool(name="cg", bufs=3) as clgp, \
         tc.tile_pool(name="cgT", bufs=3) as clgtp, \
         tc.tile_pool(name="cps", bufs=2, space="PSUM") as clpp, \
         tc.tile_pool(name="co", bufs=3) as clop, \
         tc.tile_pool(name="swk", bufs=3) as wkpool, \
         tc.tile_pool(name="shT", bufs=3) as htpool, \
         tc.tile_pool(name="srng", bufs=3) as rpool:
        whh_sb = wpool.tile([128, KH, G], BF16, name="whh_sb")
        for k in range(KH):
            nc.gpsimd.dma_start(whh_sb[:, k, :], whhd[128 * k:128 * (k + 1), :])
        c_st = stpool.tile([Bl, H], F32, name="c_st", tag="c_st")
        nc.vector.memset(c_st[:], 0.0)
        CPB = KH // NBLK
        hT = []
        for n in range(NBLK):
            t0 = htpool.tile([128, CPB * Bl], BF16, tag=f"hT{n}", name="hT0")
            nc.vector.memset(t0[:], 0.0)
            hT.append(t0)
        gxc = [None]
        ring = [None]

        def load_gx(tt):
            gxc[0] = gxpool.tile([Bl, TC, G], BF16, tag="gx", name="gxc")
            nc.gpsimd.dma_start(gxc[0][:], gxv[:, tt:tt + TC, :])

        load_gx(0)
        for t in range(Tn):
            if t % TC == 0 and t > 0:
                load_gx(t)
            if t % RC == 0:
                ring[0] = rpool.tile([Bl, RC, H], BF16, tag="ring", name="ring")
            ps = []
            for n in range(NBLK):
                p = pspool.tile([Bl, SL], F32, tag=f"ps{n}", name=f"ps{n}")
                ps.append(p)
                for j in range(SL // 512):
                    col = SL * n + 512 * j
                    nc.tensor.matmul(
                        p[:, 512 * j:512 * (j + 1)], id_sb[:],
                        gxc[0][:, t % TC, col:col + 512],
                        start=True, stop=False, skip_group_check=True)
                for k in range(KH):
                    hsrc = hT[k // CPB]
                    hcol = (k % CPB) * Bl
                    for j in range(SL // 512):
                        col = SL * n + 512 * j
                        nc.tensor.matmul(
                            p[:, 512 * j:512 * (j + 1)],
                            hsrc[:, hcol:hcol + Bl],
                            whh_sb[:, k, col:col + 512],
                            start=False, stop=(k == KH - 1),
                            skip_group_check=True)
            sgs = []
            for n in range(NBLK):
                # phase A: sigmoid over [i|f|o], tanh over g, cell update.
                # ACT/DVE queues are strict FIFO on hardware, so emit both
                # blocks' gate activations before any tanh(c) — tanh(c) of
                # block 0 depends on the DVE chain and would head-of-line
                # block sigma of block 1.
                sg = wkpool.tile([Bl, SL], BF16, tag=f"sg{n}", name="sg")
                nc.scalar.activation(sg[:, 0:3 * HB], ps[n][:, 0:3 * HB],
                                     AF.Sigmoid)
                nc.scalar.activation(sg[:, 3 * HB:4 * HB],
                                     ps[n][:, 3 * HB:4 * HB], AF.Tanh)
                cs = c_st[:, HB * n:HB * (n + 1)]
                t1 = wkpool.tile([Bl, HB], F32, tag=f"t1{n}", name="t1")
                nc.vector.tensor_tensor(out=t1[:], in0=sg[:, HB:2 * HB],
                                        in1=cs, op=ALU.mult)
                t2 = wkpool.tile([Bl, HB], F32, tag=f"t2{n}", name="t2")
                nc.vector.tensor_tensor(out=t2[:], in0=sg[:, 0:HB],
                                        in1=sg[:, 3 * HB:4 * HB], op=ALU.mult)
                nc.vector.tensor_tensor(out=cs, in0=t1[:], in1=t2[:], op=ALU.add)
                sgs.append(sg)
            for n in range(NBLK):
                # phase B: h production + transpose back for the next step
                sg = sgs[n]
                cs = c_st[:, HB * n:HB * (n + 1)]
                tch = wkpool.tile([Bl, HB], BF16, tag=f"tch{n}", name="tch")
                nc.scalar.activation(tch[:], cs, AF.Tanh)
                nc.vector.tensor_tensor(
                    out=ring[0][:, t % RC, HB * n:HB * (n + 1)],
                    in0=sg[:, 2 * HB:3 * HB], in1=tch[:], op=ALU.mult)
                hT_ps = tppool.tile([128, CPB * Bl], F32, tag="hTp", name="hT_ps")
                for kk in range(CPB):
                    lo = HB * n + 128 * kk
                    nc.tensor.matmul(
                        hT_ps[:, Bl * kk:Bl * (kk + 1)],
                        ring[0][:, t % RC, lo:lo + 128], id_sb[:],
                        start=True, stop=True)
                hTn = htpool.tile([128, CPB * Bl], BF16, tag=f"hT{n}", name="hTn")
                nc.vector.tensor_copy(hTn[:], hT_ps[:])
                hT[n] = hTn
            if (t + 1) % RC == 0:
                t0r = t + 1 - RC
                nc.gpsimd.dma_start(houtv[:, t0r:t0r + RC, :], ring[0][:])
                if cls is not None:
                    wcls_sb, logitsT = cls
                    c = t0r // RC
                    o2 = clgp.tile([128, H], BF16, tag="clin")
                    nc.scalar.dma_start(o2[:], hout[128 * c:128 * (c + 1), :])
                    o2T = clgtp.tile([128, KH, 128], BF16, tag="clinT")
                    for k in range(KH):
                        nc.sync.dma_start_transpose(
                            o2T[:, k, :], o2[:, 128 * k:128 * (k + 1)])
                    cps = clpp.tile([TAGS, 128], F32, name="cps_t")
                    for k in range(KH):
                        nc.tensor.matmul(cps[:], wcls_sb[:, k, :], o2T[:, k, :],
                                         start=(k == 0), stop=(k == KH - 1))
                    lg = clop.tile([TAGS, 128], F32, tag="cllg")
                    nc.scalar.activation(lg[:], cps[:], AF.Copy)
                    nc.scalar.dma_start(logitsT[:, 128 * c:128 * (c + 1)], lg[:])


def _prep_inputs(inputs, Tn=T, Bl=BL, ncores=NC):
    x = np.asarray(inputs["x"]).astype(np.int32)
    lengths = np.asarray(inputs["lengths"]).astype(np.int32)
    emb = np.asarray(inputs["emb"], dtype=np.float32)
    ntok = Bl * Tn
    NP = ntok + 128
    ZF = ntok          # zero row in local / f-half of h1_all
    ZB = NP + ntok     # zero row in b-half of h1_all

    com = {"ident": np.eye(Bl, dtype=ml_dtypes.bfloat16)}

    def prep_dir(fwd):
        d = {}
        for lyr, (si, sh, sb) in {1: ("W_ih_f1", "W_hh_f1", "b_f1") if fwd else
                                     ("W_ih_b1", "W_hh_b1", "b_b1"),
                                  2: ("W_ih_f2", "W_hh_f2", "b_f2") if fwd else
                                     ("W_ih_b2", "W_hh_b2", "b_b2")}.items():
            w_ih = np.asarray(inputs[si], np.float32)[_GPERM]
            w_hh = np.asarray(inputs[sh], np.float32)[_GPERM]
            bb = np.asarray(inputs[sb], np.float32)[_GPERM]
            d[f"wihT_{lyr}"] = np.ascontiguousarray(w_ih.T).astype(ml_dtypes.bfloat16)
            d[f"whhT_{lyr}"] = np.ascontiguousarray(w_hh.T).astype(ml_dtypes.bfloat16)
            d[f"bias_{lyr}"] = np.tile(bb.reshape(1, G), (128, 1)).astype(np.float32)
        wc = np.asarray(inputs["W_cls"], np.float32)  # [TAGS, 2H]
        half = wc[:, :H] if fwd else wc[:, H:]
        d["wclsT"] = np.ascontiguousarray(half.T).astype(ml_dtypes.bfloat16)
        return d

    dir_maps = {True: prep_dir(True), False: prep_dir(False)}

    def chunked(a2d):  # [Bl, Tn] -> t-major flat -> [128, ntok//128]
        a = np.ascontiguousarray(a2d.T).reshape(-1)
        return np.ascontiguousarray(a.reshape(ntok // 128, 128).T)

    in_maps = []
    for c in range(ncores):
        g = c % 4
        fwd = c < 4
        xs = x[Bl * g:Bl * (g + 1), :Tn]
        ls = np.minimum(lengths[Bl * g:Bl * (g + 1)], Tn)
        ts = np.arange(Tn)[None, :]
        valid = ts < ls[:, None]
        rev = np.where(valid, ls[:, None] - 1 - ts, ts)    # [Bl,Tn]

        bcol = np.arange(Bl)[:, None]  # t-major: row(b, t) = t*Bl + b
        if fwd:
            x_ids = xs                                 # scan order = natural
            # proj2 token (b,t): f-part row (b,t), b-part row NP + (b, rev t)
            p2f = np.where(valid, ts * Bl + bcol, ZF)
            p2b = np.where(valid, NP + rev * Bl + bcol, ZB)
        else:
            x_ids = np.take_along_axis(xs, rev, axis=1)  # scan order = reversed
            # scan position s corresponds to original t = len-1-s (s<len).
            # input token at s: f-part row (b, len-1-s) = (b, rev s) in f half,
            # b-part row (b, s) in b half.
            p2f = np.where(valid, rev * Bl + bcol, ZF)
            p2b = np.where(valid, NP + ts * Bl + bcol, ZB)

        # host-side embedding gather, t-major scan order, bf16
        e_rows = emb[np.ascontiguousarray(x_ids.T).reshape(-1)]
        m = {
            "e_rows": np.ascontiguousarray(e_rows).astype(ml_dtypes.bfloat16),
            "p2f_idx": chunked(p2f.astype(np.int32)),
            "p2b_idx": chunked(p2b.astype(np.int32)),
        }
        m.update(com)
        m.update(dir_maps[fwd])
        in_maps.append(m)
    return in_maps


_CACHED = {}


def kernel(**inputs) -> np.ndarray:
    if "nc" not in _CACHED:
        nc = bacc.Bacc("TRN2", target_bir_lowering=False, debug=False,
                       num_devices=NC)
        _build(nc)
        nc.compile()
        _CACHED["nc"] = nc
    nc = _CACHED["nc"]
    in_maps = _prep_inputs(inputs)
    res = run_bass_kernel_spmd(nc, in_maps, core_ids=list(range(NC)), trace=False)
    lengths = np.minimum(np.asarray(inputs["lengths"]).astype(np.int64), T)
    b_cls = np.asarray(inputs["b_cls"], np.float32)
    ts = np.arange(T)[None, :]
    outs = []
    for g in range(4):
        ls = lengths[BL * g:BL * (g + 1)]
        valid = (ts < ls[:, None])[:, :, None]
        rev = np.where(ts < ls[:, None], ls[:, None] - 1 - ts, 0)
        lf = res.results[g]["logitsT"].astype(np.float32)
        lb = res.results[g + 4]["logitsT"].astype(np.float32)
        Lf = np.ascontiguousarray(lf.T).reshape(T, BL, TAGS).transpose(1, 0, 2)
        Lb = np.ascontiguousarray(lb.T).reshape(T, BL, TAGS).transpose(1, 0, 2)
        Lb_nat = np.take_along_axis(Lb, rev[:, :, None], axis=1)
        outs.append(np.where(valid, Lf + Lb_nat, 0.0) + b_cls)
    return np.concatenate(outs, axis=0).astype(np.float32)


# revision 7
# speedup vs baseline: 1.8554x; 1.8554x over previous
"""BiLSTM tagger kernel for 8 Trainium2 NeuronCores — direction-sharded.

Cores 0-3 run the forward direction for sequence groups of 32; cores 4-7
run the backward direction for the same groups. Each core therefore runs
ONE lstm scan per layer over 32 sequences, so the W_hh stream (the PE
bottleneck) is amortized over twice the batch of the old 16-seq/2-scan
layout. Between layers the f/b core pairs exchange hidden states with a
pairwise AllGather; the classifier is computed as two partial products
(W_cls split by direction) summed on the host.

Gate layout is hidden-block permuted: slice n in {0,1} holds columns
[i_n | f_n | o_n | g_n] for hidden cols [256n, 256n+256), so each block's
activation/elementwise tail only depends on its own psum slice and the
next step's k-chunk matmuls can start as soon as that block's h lands.
All gates go through one sigmoid pass (W_g and b_g are pre-scaled by 2 on
the host; tanh(g) = 2*sigmoid(2g) - 1 is recovered on VectorE).

Sequence-length masking is free: scan outputs are written unmasked and
consumers (layer-2 projection, classifier) gather rows through
host-computed indices that redirect out-of-range positions to a zeroed
pad row.
"""

import sys

for _p in ("/opt/trn_rl_repo",):
    if _p not in sys.path:
        sys.path.append(_p)

import numpy as np
import ml_dtypes

import concourse.bass as bass
import concourse.tile as tile
from concourse import bacc, mybir
from concourse.bass import IndirectOffsetOnAxis
from concourse.bass_utils import run_bass_kernel_spmd

F32 = mybir.dt.float32
BF16 = mybir.dt.bfloat16
I32 = mybir.dt.int32
AF = mybir.ActivationFunctionType
ALU = mybir.AluOpType

B, T, V, E, H, TAGS = 128, 512, 50000, 256, 512, 64
NC = 8
BL = 32            # sequences per core (one direction)
G = 4 * H          # 2048 gate cols
NBLK = 2           # hidden blocks per step
HB = H // NBLK     # 256 hidden cols per block
SL = G // NBLK     # 1024 gate cols per block-slice
KH = H // 128      # 4 k-chunks for the recurrent contraction

# permuted gate order: block n -> [i_n | f_n | o_n | g_n], each HB wide
_QGATE = [0, 1, 3, 2]  # pytorch i,f,g,o -> i,f,o,g
_GPERM = np.concatenate([
    np.arange(q * H + HB * n, q * H + HB * (n + 1))
    for n in range(NBLK) for q in _QGATE])


def _build(nc, Tn=T, Bl=BL, TC=2, RC=4):
    ntok = Bl * Tn
    nchunk = ntok // 128
    NP = ntok + 128            # rows incl. zero pad block
    KE = E // 128              # 2
    KH2 = 2 * H // 128         # 8
    assert ntok % 128 == 0

    e_rows = nc.dram_tensor("e_rows", [ntok, E], BF16, kind="ExternalInput")
    p2f_idx = nc.dram_tensor("p2f_idx", [128, nchunk], I32, kind="ExternalInput")
    p2b_idx = nc.dram_tensor("p2b_idx", [128, nchunk], I32, kind="ExternalInput")
    ident = nc.dram_tensor("ident", [Bl, Bl], BF16, kind="ExternalInput")

    wih1 = nc.dram_tensor("wihT_1", [E, G], BF16, kind="ExternalInput")
    wih2 = nc.dram_tensor("wihT_2", [2 * H, G], BF16, kind="ExternalInput")
    whh = {1: nc.dram_tensor("whhT_1", [H, G], BF16, kind="ExternalInput"),
           2: nc.dram_tensor("whhT_2", [H, G], BF16, kind="ExternalInput")}
    biasd = {1: nc.dram_tensor("bias_1", [128, G], F32, kind="ExternalInput"),
             2: nc.dram_tensor("bias_2", [128, G], F32, kind="ExternalInput")}
    wcls = nc.dram_tensor("wclsT", [H, TAGS], BF16, kind="ExternalInput")

    gx = {1: nc.dram_tensor("gx_1", [ntok, G], BF16),
          2: nc.dram_tensor("gx_2", [ntok, G], BF16)}
    h1 = nc.dram_tensor("h1", [NP, H], BF16)
    h1_all = nc.dram_tensor("h1_all", [2 * NP, H], BF16)
    h2 = nc.dram_tensor("h2", [NP, H], BF16)
    logitsT = nc.dram_tensor("logitsT", [TAGS, ntok], F32, kind="ExternalOutput")

    with tile.TileContext(nc) as tc:
        with tc.tile_pool(name="const", bufs=1) as cpool:
            def load_const(nm, shape, dt, src_ap):
                t = cpool.tile(shape, dt, name=nm, tag=nm)
                nc.gpsimd.dma_start(t[:], src_ap)
                return t

            p2f_sb = load_const("p2f_sb", [128, nchunk], I32, p2f_idx[:])
            p2b_sb = load_const("p2b_sb", [128, nchunk], I32, p2b_idx[:])
            id_sb = load_const("id_sb", [Bl, Bl], BF16, ident[:])
            bias_sb = {l: load_const(f"bias_sb_{l}", [128, G], F32, biasd[l][:])
                       for l in (1, 2)}
            wcls_sb = cpool.tile([128, KH, TAGS], BF16, name="wcls_sb")
            for k in range(KH):
                nc.gpsimd.dma_start(wcls_sb[:, k, :], wcls[128 * k:128 * (k + 1), :])

            # zero the pad row blocks of h1/h2
            zpad = cpool.tile([128, H], BF16, name="zpad", tag="zpad")
            nc.vector.memset(zpad[:], 0.0)
            nc.gpsimd.dma_start(h1[ntok:NP, :], zpad[:])
            nc.gpsimd.dma_start(h2[ntok:NP, :], zpad[:])

            # ---- layer 1: projection fused under the scan (t-major gx:
            # chunk c feeds scan steps 4c..4c+4, emitted PRE chunks ahead) ----
            _scan_phase(nc, tc, Tn, Bl, TC, RC, whh[1], gx[1], h1, id_sb,
                        proj=(KE, wih1, bias_sb[1], ("rows", e_rows, None)))

            # ---- exchange hidden states between f/b core pairs ----
            nc.gpsimd.collective_compute(
                "AllGather", ALU.bypass,
                replica_groups=[[0, 4], [1, 5], [2, 6], [3, 7]],
                ins=[h1[:].opt()], outs=[h1_all[:].opt()])

            # ---- layer 2 scan with projection AND the partial classifier fused in:
            # each ring flush covers exactly one 128-row t-major chunk of h2,
            # so the classifier chunk runs under the scan instead of as a
            # 1ms latency-bound tail phase ----
            _scan_phase(nc, tc, Tn, Bl, TC, RC, whh[2], gx[2], h2, id_sb,
                        cls=(wcls_sb, logitsT),
                        proj=(KH2, wih2, bias_sb[2],
                              ("pair", h1_all, (p2f_sb, p2b_sb))))
    return nc


def _proj_phase(nc, tc, nchunk, KD, wih, bias_sb, gxd, src):
    """gx = input @ wih + bias, token chunks of 128 in scan order."""
    D = KD * 128
    kind, dsrc, idx = src
    with tc.tile_pool(name="pw", bufs=1) as wpool, \
         tc.tile_pool(name="pg", bufs=3) as gpool, \
         tc.tile_pool(name="pgT", bufs=3) as tpool, \
         tc.tile_pool(name="pps", bufs=4, space="PSUM") as ppool, \
         tc.tile_pool(name="pout", bufs=3) as opool:
        wsb = wpool.tile([128, KD, G], BF16, name="wih_sb")
        for k in range(KD):
            nc.gpsimd.dma_start(wsb[:, k, :], wih[128 * k:128 * (k + 1), :])
        for c in range(nchunk):
            if kind == "rows":
                xin = gpool.tile([128, D], BF16, tag="e16")
                nc.gpsimd.dma_start(xin[:], dsrc[128 * c:128 * (c + 1), :])
            else:
                fidx, bidx = idx
                xin = gpool.tile([128, D], BF16, tag="e16")
                nc.gpsimd.indirect_dma_start(
                    out=xin[:, 0:D // 2], out_offset=None, in_=dsrc[:],
                    in_offset=IndirectOffsetOnAxis(ap=fidx[:, c:c + 1], axis=0))
                nc.gpsimd.indirect_dma_start(
                    out=xin[:, D // 2:D], out_offset=None, in_=dsrc[:],
                    in_offset=IndirectOffsetOnAxis(ap=bidx[:, c:c + 1], axis=0))
            xT = tpool.tile([128, KD, 128], BF16, tag="xT")
            for k in range(KD):
                nc.sync.dma_start_transpose(
                    xT[:, k, :], xin[:, 128 * k:128 * (k + 1)])
            gout = opool.tile([128, G], BF16, tag="gout")
            for n in range(G // 512):
                ps = ppool.tile([128, 512], F32, name="pps")
                for k in range(KD):
                    nc.tensor.matmul(
                        ps[:], xT[:, k, :], wsb[:, k, 512 * n:512 * (n + 1)],
                        start=(k == 0), stop=(k == KD - 1))
                nc.vector.tensor_tensor(
                    out=gout[:, 512 * n:512 * (n + 1)], in0=ps[:],
                    in1=bias_sb[:, 512 * n:512 * (n + 1)], op=ALU.add)
            nc.gpsimd.dma_start(gxd[128 * c:128 * (c + 1), :], gout[:])


def _scan_phase(nc, tc, Tn, Bl, TC, RC, whhd, gxd, hout, id_sb, cls=None,
                proj=None):
    """One-direction scan over Bl sequences; hidden-block pipelined."""
    # token rows are TIME-MAJOR: row = t*Bl + b, so layer-2 projection
    # chunks stream in time order and overlap under the scan
    gxv = gxd.ap().rearrange("(t b) d -> b t d", b=Bl)
    houtv = hout.ap()[0:Bl * Tn, :].rearrange("(t b) d -> b t d", b=Bl)
    with tc.tile_pool(name="sw", bufs=1) as wpool, \
         tc.tile_pool(name="sgx", bufs=4) as gxpool, \
         tc.tile_pool(name="sst", bufs=1) as stpool, \
         tc.tile_pool(name="sps", bufs=1, space="PSUM") as pspool, \
         tc.tile_pool(name="spsT", bufs=2, space="PSUM") as tppool, \
         tc.tile_pool(name="cg", bufs=3) as clgp, \
         tc.tile_pool(name="cgT", bufs=3) as clgtp, \
         tc.tile_pool(name="cps", bufs=1, space="PSUM") as clpp, \
         tc.tile_pool(name="pw", bufs=1) as pwpool, \
         tc.tile_pool(name="pg", bufs=3) as pgpool, \
         tc.tile_pool(name="pgT", bufs=3) as ptpool, \
         tc.tile_pool(name="pps", bufs=1, space="PSUM") as pppool, \
         tc.tile_pool(name="pout", bufs=3) as popool, \
         tc.tile_pool(name="co", bufs=3) as clop, \
         tc.tile_pool(name="swk", bufs=3) as wkpool, \
         tc.tile_pool(name="shT", bufs=3) as htpool, \
         tc.tile_pool(name="srng", bufs=3) as rpool:
        whh_sb = wpool.tile([128, KH, G], BF16, name="whh_sb")
        for k in range(KH):
            nc.gpsimd.dma_start(whh_sb[:, k, :], whhd[128 * k:128 * (k + 1), :])
        c_st = stpool.tile([Bl, H], F32, name="c_st", tag="c_st")
        nc.vector.memset(c_st[:], 0.0)
        CPB = KH // NBLK
        hT = []
        for n in range(NBLK):
            t0 = htpool.tile([128, CPB * Bl], BF16, tag=f"hT{n}", name="hT0")
            nc.vector.memset(t0[:], 0.0)
            hT.append(t0)
        gxc = [None]
        ring = [None]
        nchunk = Bl * Tn // 128
        PRE = 6  # proj chunks emitted ahead of the consuming scan steps

        if proj is not None:
            KD, wih, bias_sb, psrc = proj
            D = KD * 128
            pkind, pdsrc, pidx = psrc
            wsb = pwpool.tile([128, KD, G], BF16, name="wih_sb")
            for k in range(KD):
                nc.gpsimd.dma_start(wsb[:, k, :], wih[128 * k:128 * (k + 1), :])

        def emit_proj_chunk(c):
            if proj is None or c >= nchunk:
                return
            if pkind == "rows":
                xin = pgpool.tile([128, D], BF16, tag="e16")
                nc.gpsimd.dma_start(xin[:], pdsrc[128 * c:128 * (c + 1), :])
            else:
                fidx, bidx = pidx
                xin = pgpool.tile([128, D], BF16, tag="e16")
                nc.gpsimd.indirect_dma_start(
                    out=xin[:, 0:D // 2], out_offset=None, in_=pdsrc[:],
                    in_offset=IndirectOffsetOnAxis(ap=fidx[:, c:c + 1], axis=0))
                nc.gpsimd.indirect_dma_start(
                    out=xin[:, D // 2:D], out_offset=None, in_=pdsrc[:],
                    in_offset=IndirectOffsetOnAxis(ap=bidx[:, c:c + 1], axis=0))
            xT = ptpool.tile([128, KD, 128], BF16, tag="xT")
            for k in range(KD):
                nc.sync.dma_start_transpose(
                    xT[:, k, :], xin[:, 128 * k:128 * (k + 1)])
            gout = popool.tile([128, G], BF16, tag="gout")
            for n in range(G // 512):
                pp = pppool.tile([128, 512], F32, name="pps")
                for k in range(KD):
                    nc.tensor.matmul(
                        pp[:], xT[:, k, :], wsb[:, k, 512 * n:512 * (n + 1)],
                        start=(k == 0), stop=(k == KD - 1))
                nc.vector.tensor_tensor(
                    out=gout[:, 512 * n:512 * (n + 1)], in0=pp[:],
                    in1=bias_sb[:, 512 * n:512 * (n + 1)], op=ALU.add)
            nc.gpsimd.dma_start(gxd[128 * c:128 * (c + 1), :], gout[:])

        def load_gx(tt):
            gxc[0] = gxpool.tile([Bl, TC, G], BF16, tag="gx", name="gxc")
            nc.gpsimd.dma_start(gxc[0][:], gxv[:, tt:tt + TC, :])

        for c in range(PRE):
            emit_proj_chunk(c)
        load_gx(0)
        for t in range(Tn):
            if t % TC == 0 and t > 0:
                load_gx(t)
            if t % RC == 0:
                ring[0] = rpool.tile([Bl, RC, H], BF16, tag="ring", name="ring")
                emit_proj_chunk(t // RC + PRE)
            ps = []
            for n in range(NBLK):
                p = pspool.tile([Bl, SL], F32, tag=f"ps{n}", name=f"ps{n}")
                ps.append(p)
                for j in range(SL // 512):
                    col = SL * n + 512 * j
                    nc.tensor.matmul(
                        p[:, 512 * j:512 * (j + 1)], id_sb[:],
                        gxc[0][:, t % TC, col:col + 512],
                        start=True, stop=False, skip_group_check=True)
                for k in range(KH):
                    hsrc = hT[k // CPB]
                    hcol = (k % CPB) * Bl
                    for j in range(SL // 512):
                        col = SL * n + 512 * j
                        nc.tensor.matmul(
                            p[:, 512 * j:512 * (j + 1)],
                            hsrc[:, hcol:hcol + Bl],
                            whh_sb[:, k, col:col + 512],
                            start=False, stop=(k == KH - 1),
                            skip_group_check=True)
            sgs = []
            for n in range(NBLK):
                # phase A: sigmoid over [i|f|o], tanh over g, cell update.
                # ACT/DVE queues are strict FIFO on hardware, so emit both
                # blocks' gate activations before any tanh(c) — tanh(c) of
                # block 0 depends on the DVE chain and would head-of-line
                # block sigma of block 1.
                sg = wkpool.tile([Bl, SL], BF16, tag=f"sg{n}", name="sg")
                nc.scalar.activation(sg[:, 0:3 * HB], ps[n][:, 0:3 * HB],
                                     AF.Sigmoid)
                nc.scalar.activation(sg[:, 3 * HB:4 * HB],
                                     ps[n][:, 3 * HB:4 * HB], AF.Tanh)
                cs = c_st[:, HB * n:HB * (n + 1)]
                t1 = wkpool.tile([Bl, HB], F32, tag=f"t1{n}", name="t1")
                nc.vector.tensor_tensor(out=t1[:], in0=sg[:, HB:2 * HB],
                                        in1=cs, op=ALU.mult)
                t2 = wkpool.tile([Bl, HB], F32, tag=f"t2{n}", name="t2")
                nc.vector.tensor_tensor(out=t2[:], in0=sg[:, 0:HB],
                                        in1=sg[:, 3 * HB:4 * HB], op=ALU.mult)
                nc.vector.tensor_tensor(out=cs, in0=t1[:], in1=t2[:], op=ALU.add)
                sgs.append(sg)
            for n in range(NBLK):
                # phase B: h production + transpose back for the next step
                sg = sgs[n]
                cs = c_st[:, HB * n:HB * (n + 1)]
                tch = wkpool.tile([Bl, HB], BF16, tag=f"tch{n}", name="tch")
                nc.scalar.activation(tch[:], cs, AF.Tanh)
                nc.vector.tensor_tensor(
                    out=ring[0][:, t % RC, HB * n:HB * (n + 1)],
                    in0=sg[:, 2 * HB:3 * HB], in1=tch[:], op=ALU.mult)
                hT_ps = tppool.tile([128, CPB * Bl], F32, tag="hTp", name="hT_ps")
                for kk in range(CPB):
                    lo = HB * n + 128 * kk
                    nc.tensor.matmul(
                        hT_ps[:, Bl * kk:Bl * (kk + 1)],
                        ring[0][:, t % RC, lo:lo + 128], id_sb[:],
                        start=True, stop=True)
                hTn = htpool.tile([128, CPB * Bl], BF16, tag=f"hT{n}", name="hTn")
                nc.vector.tensor_copy(hTn[:], hT_ps[:])
                hT[n] = hTn
            if (t + 1) % RC == 0:
                t0r = t + 1 - RC
                nc.gpsimd.dma_start(houtv[:, t0r:t0r + RC, :], ring[0][:])
                if cls is not None:
                    wcls_sb, logitsT = cls
                    c = t0r // RC
                    o2 = clgp.tile([128, H], BF16, tag="clin")
                    nc.scalar.dma_start(o2[:], hout[128 * c:128 * (c + 1), :])
                    o2T = clgtp.tile([128, KH, 128], BF16, tag="clinT")
                    for k in range(KH):
                        nc.sync.dma_start_transpose(
                            o2T[:, k, :], o2[:, 128 * k:128 * (k + 1)])
                    cps = clpp.tile([TAGS, 128], F32, name="cps_t")
                    for k in range(KH):
                        nc.tensor.matmul(cps[:], wcls_sb[:, k, :], o2T[:, k, :],
                                         start=(k == 0), stop=(k == KH - 1))
                    lg = clop.tile([TAGS, 128], F32, tag="cllg")
                    nc.scalar.activation(lg[:], cps[:], AF.Copy)
                    nc.scalar.dma_start(logitsT[:, 128 * c:128 * (c + 1)], lg[:])


def _prep_inputs(inputs, Tn=T, Bl=BL, ncores=NC):
    x = np.asarray(inputs["x"]).astype(np.int32)
    lengths = np.asarray(inputs["lengths"]).astype(np.int32)
    emb = np.asarray(inputs["emb"], dtype=np.float32)
    ntok = Bl * Tn
    NP = ntok + 128
    ZF = ntok          # zero row in local / f-half of h1_all
    ZB = NP + ntok     # zero row in b-half of h1_all

    com = {"ident": np.eye(Bl, dtype=ml_dtypes.bfloat16)}

    def prep_dir(fwd):
        d = {}
        for lyr, (si, sh, sb) in {1: ("W_ih_f1", "W_hh_f1", "b_f1") if fwd else
                                     ("W_ih_b1", "W_hh_b1", "b_b1"),
                                  2: ("W_ih_f2", "W_hh_f2", "b_f2") if fwd else
                                     ("W_ih_b2", "W_hh_b2", "b_b2")}.items():
            w_ih = np.asarray(inputs[si], np.float32)[_GPERM]
            w_hh = np.asarray(inputs[sh], np.float32)[_GPERM]
            bb = np.asarray(inputs[sb], np.float32)[_GPERM]
            d[f"wihT_{lyr}"] = np.ascontiguousarray(w_ih.T).astype(ml_dtypes.bfloat16)
            d[f"whhT_{lyr}"] = np.ascontiguousarray(w_hh.T).astype(ml_dtypes.bfloat16)
            d[f"bias_{lyr}"] = np.tile(bb.reshape(1, G), (128, 1)).astype(np.float32)
        wc = np.asarray(inputs["W_cls"], np.float32)  # [TAGS, 2H]
        half = wc[:, :H] if fwd else wc[:, H:]
        d["wclsT"] = np.ascontiguousarray(half.T).astype(ml_dtypes.bfloat16)
        return d

    dir_maps = {True: prep_dir(True), False: prep_dir(False)}

    def chunked(a2d):  # [Bl, Tn] -> t-major flat -> [128, ntok//128]
        a = np.ascontiguousarray(a2d.T).reshape(-1)
        return np.ascontiguousarray(a.reshape(ntok // 128, 128).T)

    in_maps = []
    for c in range(ncores):
        g = c % 4
        fwd = c < 4
        xs = x[Bl * g:Bl * (g + 1), :Tn]
        ls = np.minimum(lengths[Bl * g:Bl * (g + 1)], Tn)
        ts = np.arange(Tn)[None, :]
        valid = ts < ls[:, None]
        rev = np.where(valid, ls[:, None] - 1 - ts, ts)    # [Bl,Tn]

        bcol = np.arange(Bl)[:, None]  # t-major: row(b, t) = t*Bl + b
        if fwd:
            x_ids = xs                                 # scan order = natural
            # proj2 token (b,t): f-part row (b,t), b-part row NP + (b, rev t)
            p2f = np.where(valid, ts * Bl + bcol, ZF)
            p2b = np.where(valid, NP + rev * Bl + bcol, ZB)
        else:
            x_ids = np.take_along_axis(xs, rev, axis=1)  # scan order = reversed
            # scan position s corresponds to original t = len-1-s (s<len).
            # input token at s: f-part row (b, len-1-s) = (b, rev s) in f half,
            # b-part row (b, s) in b half.
            p2f = np.where(valid, rev * Bl + bcol, ZF)
            p2b = np.where(valid, NP + ts * Bl + bcol, ZB)

        # host-side embedding gather, t-major scan order, bf16
        e_rows = emb[np.ascontiguousarray(x_ids.T).reshape(-1)]
        m = {
            "e_rows": np.ascontiguousarray(e_rows).astype(ml_dtypes.bfloat16),
            "p2f_idx": chunked(p2f.astype(np.int32)),
            "p2b_idx": chunked(p2b.astype(np.int32)),
        }
        m.update(com)
        m.update(dir_maps[fwd])
        in_maps.append(m)
    return in_maps


_CACHED = {}


def kernel(**inputs) -> np.ndarray:
    if "nc" not in _CACHED:
        nc = bacc.Bacc("TRN2", target_bir_lowering=False, debug=False,
                       num_devices=NC)
        _build(nc)
        nc.compile()
        _CACHED["nc"] = nc
    nc = _CACHED["nc"]
    in_maps = _prep_inputs(inputs)
    res = run_bass_kernel_spmd(nc, in_maps, core_ids=list(range(NC)), trace=False)
    lengths = np.minimum(np.asarray(inputs["lengths"]).astype(np.int64), T)
    b_cls = np.asarray(inputs["b_cls"], np.float32)
    ts = np.arange(T)[None, :]
    outs = []
    for g in range(4):
        ls = lengths[BL * g:BL * (g + 1)]
        valid = (ts < ls[:, None])[:, :, None]
        rev = np.where(ts < ls[:, None], ls[:, None] - 1 - ts, 0)
        lf = res.results[g]["logitsT"].astype(np.float32)
        lb = res.results[g + 4]["logitsT"].astype(np.float32)
        Lf = np.ascontiguousarray(lf.T).reshape(T, BL, TAGS).transpose(1, 0, 2)
        Lb = np.ascontiguousarray(lb.T).reshape(T, BL, TAGS).transpose(1, 0, 2)
        Lb_nat = np.take_along_axis(Lb, rev[:, :, None], axis=1)
        outs.append(np.where(valid, Lf + Lb_nat, 0.0) + b_cls)
    return np.concatenate(outs, axis=0).astype(np.float32)


# revision 11
# speedup vs baseline: 1.8767x; 1.0115x over previous
"""BiLSTM tagger kernel for 8 Trainium2 NeuronCores — direction-sharded.

Cores 0-3 run the forward direction for sequence groups of 32; cores 4-7
run the backward direction for the same groups. Each core therefore runs
ONE lstm scan per layer over 32 sequences, so the W_hh stream (the PE
bottleneck) is amortized over twice the batch of the old 16-seq/2-scan
layout. Between layers the f/b core pairs exchange hidden states with a
pairwise AllGather; the classifier is computed as two partial products
(W_cls split by direction) summed on the host.

Gate layout is hidden-block permuted: slice n in {0,1} holds columns
[i_n | f_n | o_n | g_n] for hidden cols [256n, 256n+256), so each block's
activation/elementwise tail only depends on its own psum slice. Per block:
one sigmoid over [i|f|o] and one tanh over g, both reading straight from
PSUM (gx was pre-injected there via identity matmuls); tail emission is
split into two phases because the hardware ACT/DVE queues are strict FIFO.

Token rows are time-major (row = t*32 + b), which lines up producer and
consumer dram ranges chunk-for-chunk, so the input projections AND the
partial classifier are emitted inside the scan loops and hide under the
recurrent matmul stream. Sequence-length masking is free: out-of-range
positions gather from a zeroed pad row, and the host applies the final
mask/bias. Embeddings are gathered on the host (16.8MB of bf16 rows per
core instead of a replicated 51MB table).
"""

import sys

for _p in ("/opt/trn_rl_repo",):
    if _p not in sys.path:
        sys.path.append(_p)

import numpy as np
import ml_dtypes

import concourse.bass as bass
import concourse.tile as tile
from concourse import bacc, mybir
from concourse.bass import IndirectOffsetOnAxis
from concourse.bass_utils import run_bass_kernel_spmd

F32 = mybir.dt.float32
BF16 = mybir.dt.bfloat16
I32 = mybir.dt.int32
AF = mybir.ActivationFunctionType
ALU = mybir.AluOpType

B, T, V, E, H, TAGS = 128, 512, 50000, 256, 512, 64
NC = 8
BL = 32            # sequences per core (one direction)
G = 4 * H          # 2048 gate cols
NBLK = 4           # hidden blocks per step
HB = H // NBLK     # 256 hidden cols per block
SL = G // NBLK     # 1024 gate cols per block-slice
KH = H // 128      # 4 k-chunks for the recurrent contraction

# permuted gate order: block n -> [i_n | f_n | o_n | g_n], each HB wide
_QGATE = [0, 1, 3, 2]  # pytorch i,f,g,o -> i,f,o,g
_GPERM = np.concatenate([
    np.arange(q * H + HB * n, q * H + HB * (n + 1))
    for n in range(NBLK) for q in _QGATE])


def _build(nc, Tn=T, Bl=BL, TC=2, RC=4):
    ntok = Bl * Tn
    nchunk = ntok // 128
    NP = ntok + 128            # rows incl. zero pad block
    KE = E // 128              # 2
    KH2 = 2 * H // 128         # 8
    assert ntok % 128 == 0

    e_rows = nc.dram_tensor("e_rows", [ntok, E], BF16, kind="ExternalInput")
    p2f_idx = nc.dram_tensor("p2f_idx", [128, nchunk], I32, kind="ExternalInput")
    p2b_idx = nc.dram_tensor("p2b_idx", [128, nchunk], I32, kind="ExternalInput")
    ident = nc.dram_tensor("ident", [Bl, Bl], BF16, kind="ExternalInput")

    wih1 = nc.dram_tensor("wihT_1", [E, G], BF16, kind="ExternalInput")
    wih2 = nc.dram_tensor("wihT_2", [2 * H, G], BF16, kind="ExternalInput")
    whh = {1: nc.dram_tensor("whhT_1", [H, G], BF16, kind="ExternalInput"),
           2: nc.dram_tensor("whhT_2", [H, G], BF16, kind="ExternalInput")}
    biasd = {1: nc.dram_tensor("bias_1", [128, G], F32, kind="ExternalInput"),
             2: nc.dram_tensor("bias_2", [128, G], F32, kind="ExternalInput")}
    wcls = nc.dram_tensor("wclsT", [H, TAGS], BF16, kind="ExternalInput")

    gx = {1: nc.dram_tensor("gx_1", [ntok, G], BF16),
          2: nc.dram_tensor("gx_2", [ntok, G], BF16)}
    h1 = nc.dram_tensor("h1", [NP, H], BF16)
    h1_all = nc.dram_tensor("h1_all", [2 * NP, H], BF16)
    h2 = nc.dram_tensor("h2", [NP, H], BF16)
    logitsT = nc.dram_tensor("logitsT", [TAGS, ntok], F32, kind="ExternalOutput")

    with tile.TileContext(nc) as tc:
        with tc.tile_pool(name="const", bufs=1) as cpool:
            def load_const(nm, shape, dt, src_ap):
                t = cpool.tile(shape, dt, name=nm, tag=nm)
                nc.gpsimd.dma_start(t[:], src_ap)
                return t

            p2f_sb = load_const("p2f_sb", [128, nchunk], I32, p2f_idx[:])
            p2b_sb = load_const("p2b_sb", [128, nchunk], I32, p2b_idx[:])
            id_sb = load_const("id_sb", [Bl, Bl], BF16, ident[:])
            bias_sb = {l: load_const(f"bias_sb_{l}", [128, G], F32, biasd[l][:])
                       for l in (1, 2)}
            wcls_sb = cpool.tile([128, KH, TAGS], BF16, name="wcls_sb")
            for k in range(KH):
                nc.gpsimd.dma_start(wcls_sb[:, k, :], wcls[128 * k:128 * (k + 1), :])

            # zero the pad row blocks of h1/h2
            zpad = cpool.tile([128, H], BF16, name="zpad", tag="zpad")
            nc.vector.memset(zpad[:], 0.0)
            nc.gpsimd.dma_start(h1[ntok:NP, :], zpad[:])
            nc.gpsimd.dma_start(h2[ntok:NP, :], zpad[:])

            # ---- layer 1: projection fused under the scan (t-major gx:
            # chunk c feeds scan steps 4c..4c+4, emitted PRE chunks ahead) ----
            _scan_phase(nc, tc, Tn, Bl, TC, RC, whh[1], gx[1], h1, id_sb,
                        proj=(KE, wih1, bias_sb[1], ("rows", e_rows, None)))

            # ---- exchange hidden states between f/b core pairs ----
            nc.gpsimd.collective_compute(
                "AllGather", ALU.bypass,
                replica_groups=[[0, 4], [1, 5], [2, 6], [3, 7]],
                ins=[h1[:].opt()], outs=[h1_all[:].opt()])

            # ---- layer 2 scan with projection AND the partial classifier fused in:
            # each ring flush covers exactly one 128-row t-major chunk of h2,
            # so the classifier chunk runs under the scan instead of as a
            # 1ms latency-bound tail phase ----
            _scan_phase(nc, tc, Tn, Bl, TC, RC, whh[2], gx[2], h2, id_sb,
                        cls=(wcls_sb, logitsT),
                        proj=(KH2, wih2, bias_sb[2],
                              ("pair", h1_all, (p2f_sb, p2b_sb))))
    return nc


def _proj_phase(nc, tc, nchunk, KD, wih, bias_sb, gxd, src):
    """gx = input @ wih + bias, token chunks of 128 in scan order."""
    D = KD * 128
    kind, dsrc, idx = src
    with tc.tile_pool(name="pw", bufs=1) as wpool, \
         tc.tile_pool(name="pg", bufs=3) as gpool, \
         tc.tile_pool(name="pgT", bufs=3) as tpool, \
         tc.tile_pool(name="pps", bufs=4, space="PSUM") as ppool, \
         tc.tile_pool(name="pout", bufs=3) as opool:
        wsb = wpool.tile([128, KD, G], BF16, name="wih_sb")
        for k in range(KD):
            nc.gpsimd.dma_start(wsb[:, k, :], wih[128 * k:128 * (k + 1), :])
        for c in range(nchunk):
            if kind == "rows":
                xin = gpool.tile([128, D], BF16, tag="e16")
                nc.gpsimd.dma_start(xin[:], dsrc[128 * c:128 * (c + 1), :])
            else:
                fidx, bidx = idx
                xin = gpool.tile([128, D], BF16, tag="e16")
                nc.gpsimd.indirect_dma_start(
                    out=xin[:, 0:D // 2], out_offset=None, in_=dsrc[:],
                    in_offset=IndirectOffsetOnAxis(ap=fidx[:, c:c + 1], axis=0))
                nc.gpsimd.indirect_dma_start(
                    out=xin[:, D // 2:D], out_offset=None, in_=dsrc[:],
                    in_offset=IndirectOffsetOnAxis(ap=bidx[:, c:c + 1], axis=0))
            xT = tpool.tile([128, KD, 128], BF16, tag="xT")
            for k in range(KD):
                nc.sync.dma_start_transpose(
                    xT[:, k, :], xin[:, 128 * k:128 * (k + 1)])
            gout = opool.tile([128, G], BF16, tag="gout")
            for n in range(G // 512):
                ps = ppool.tile([128, 512], F32, name="pps")
                for k in range(KD):
                    nc.tensor.matmul(
                        ps[:], xT[:, k, :], wsb[:, k, 512 * n:512 * (n + 1)],
                        start=(k == 0), stop=(k == KD - 1))
                nc.vector.tensor_tensor(
                    out=gout[:, 512 * n:512 * (n + 1)], in0=ps[:],
                    in1=bias_sb[:, 512 * n:512 * (n + 1)], op=ALU.add)
            nc.gpsimd.dma_start(gxd[128 * c:128 * (c + 1), :], gout[:])


def _scan_phase(nc, tc, Tn, Bl, TC, RC, whhd, gxd, hout, id_sb, cls=None,
                proj=None, coll=None):
    """One-direction scan over Bl sequences; hidden-block pipelined."""
    # token rows are TIME-MAJOR: row = t*Bl + b, so layer-2 projection
    # chunks stream in time order and overlap under the scan
    gxv = gxd.ap().rearrange("(t b) d -> b t d", b=Bl)
    houtv = hout.ap()[0:Bl * Tn, :].rearrange("(t b) d -> b t d", b=Bl)
    with tc.tile_pool(name="sw", bufs=1) as wpool, \
         tc.tile_pool(name="sgx", bufs=4) as gxpool, \
         tc.tile_pool(name="sst", bufs=1) as stpool, \
         tc.tile_pool(name="sps", bufs=1, space="PSUM") as pspool, \
         tc.tile_pool(name="spsT", bufs=2, space="PSUM") as tppool, \
         tc.tile_pool(name="cg", bufs=3) as clgp, \
         tc.tile_pool(name="cgT", bufs=3) as clgtp, \
         tc.tile_pool(name="cps", bufs=1, space="PSUM") as clpp, \
         tc.tile_pool(name="pw", bufs=1) as pwpool, \
         tc.tile_pool(name="pg", bufs=3) as pgpool, \
         tc.tile_pool(name="pgT", bufs=3) as ptpool, \
         tc.tile_pool(name="pps", bufs=1, space="PSUM") as pppool, \
         tc.tile_pool(name="pout", bufs=3) as popool, \
         tc.tile_pool(name="co", bufs=3) as clop, \
         tc.tile_pool(name="swk", bufs=3) as wkpool, \
         tc.tile_pool(name="shT", bufs=3) as htpool, \
         tc.tile_pool(name="srng", bufs=3) as rpool:
        whh_sb = wpool.tile([128, KH, G], BF16, name="whh_sb")
        for k in range(KH):
            nc.gpsimd.dma_start(whh_sb[:, k, :], whhd[128 * k:128 * (k + 1), :])
        c_st = stpool.tile([Bl, H], F32, name="c_st", tag="c_st")
        nc.vector.memset(c_st[:], 0.0)
        CPB = KH // NBLK
        hT = []
        for n in range(NBLK):
            t0 = htpool.tile([128, CPB * Bl], BF16, tag=f"hT{n}", name="hT0")
            nc.vector.memset(t0[:], 0.0)
            hT.append(t0)
        gxc = [None]
        ring = [None]
        nchunk = Bl * Tn // 128
        PRE = 6  # proj chunks emitted ahead of the consuming scan steps

        if proj is not None:
            KD, wih, bias_sb, psrc = proj
            D = KD * 128
            pkind, pdsrc, pidx = psrc
            wsb = pwpool.tile([128, KD, G], BF16, name="wih_sb")
            for k in range(KD):
                nc.gpsimd.dma_start(wsb[:, k, :], wih[128 * k:128 * (k + 1), :])

        def emit_proj_chunk(c):
            if proj is None or c >= nchunk:
                return
            if pkind == "rows":
                xin = pgpool.tile([128, D], BF16, tag="e16")
                nc.gpsimd.dma_start(xin[:], pdsrc[128 * c:128 * (c + 1), :])
            else:
                fidx, bidx = pidx
                xin = pgpool.tile([128, D], BF16, tag="e16")
                nc.gpsimd.indirect_dma_start(
                    out=xin[:, 0:D // 2], out_offset=None, in_=pdsrc[:],
                    in_offset=IndirectOffsetOnAxis(ap=fidx[:, c:c + 1], axis=0))
                nc.gpsimd.indirect_dma_start(
                    out=xin[:, D // 2:D], out_offset=None, in_=pdsrc[:],
                    in_offset=IndirectOffsetOnAxis(ap=bidx[:, c:c + 1], axis=0))
            xT = ptpool.tile([128, KD, 128], BF16, tag="xT")
            for k in range(KD):
                nc.sync.dma_start_transpose(
                    xT[:, k, :], xin[:, 128 * k:128 * (k + 1)])
            gout = popool.tile([128, G], BF16, tag="gout")
            for n in range(G // 512):
                pp = pppool.tile([128, 512], F32, name="pps")
                for k in range(KD):
                    nc.tensor.matmul(
                        pp[:], xT[:, k, :], wsb[:, k, 512 * n:512 * (n + 1)],
                        start=(k == 0), stop=(k == KD - 1))
                nc.vector.tensor_tensor(
                    out=gout[:, 512 * n:512 * (n + 1)], in0=pp[:],
                    in1=bias_sb[:, 512 * n:512 * (n + 1)], op=ALU.add)
            nc.gpsimd.dma_start(gxd[128 * c:128 * (c + 1), :], gout[:])

        def load_gx(tt):
            gxc[0] = gxpool.tile([Bl, TC, G], BF16, tag="gx", name="gxc")
            nc.gpsimd.dma_start(gxc[0][:], gxv[:, tt:tt + TC, :])

        for c in range(PRE):
            emit_proj_chunk(c)
        load_gx(0)
        for t in range(Tn):
            if t % TC == 0 and t > 0:
                load_gx(t)
            if t % RC == 0:
                ring[0] = rpool.tile([Bl, RC, H], BF16, tag="ring", name="ring")
                emit_proj_chunk(t // RC + PRE)
            ps = []
            for n in range(NBLK):
                p = pspool.tile([Bl, SL], F32, tag=f"ps{n}", name=f"ps{n}")
                ps.append(p)
                for j in range(SL // 512):
                    col = SL * n + 512 * j
                    nc.tensor.matmul(
                        p[:, 512 * j:512 * (j + 1)], id_sb[:],
                        gxc[0][:, t % TC, col:col + 512],
                        start=True, stop=False, skip_group_check=True)
                for k in range(KH):
                    hsrc = hT[k // CPB]
                    hcol = (k % CPB) * Bl
                    for j in range(SL // 512):
                        col = SL * n + 512 * j
                        nc.tensor.matmul(
                            p[:, 512 * j:512 * (j + 1)],
                            hsrc[:, hcol:hcol + Bl],
                            whh_sb[:, k, col:col + 512],
                            start=False, stop=(k == KH - 1),
                            skip_group_check=True)
            sgs = []
            for n in range(NBLK):
                # phase A: sigmoid over [i|f|o], tanh over g, cell update.
                # ACT/DVE queues are strict FIFO on hardware, so emit both
                # blocks' gate activations before any tanh(c) — tanh(c) of
                # block 0 depends on the DVE chain and would head-of-line
                # block sigma of block 1.
                sg = wkpool.tile([Bl, SL], BF16, tag=f"sg{n}", name="sg")
                nc.scalar.activation(sg[:, 0:3 * HB], ps[n][:, 0:3 * HB],
                                     AF.Sigmoid)
                nc.scalar.activation(sg[:, 3 * HB:4 * HB],
                                     ps[n][:, 3 * HB:4 * HB], AF.Tanh)
                cs = c_st[:, HB * n:HB * (n + 1)]
                t1 = wkpool.tile([Bl, HB], F32, tag=f"t1{n}", name="t1")
                nc.vector.tensor_tensor(out=t1[:], in0=sg[:, HB:2 * HB],
                                        in1=cs, op=ALU.mult)
                t2 = wkpool.tile([Bl, HB], F32, tag=f"t2{n}", name="t2")
                nc.vector.tensor_tensor(out=t2[:], in0=sg[:, 0:HB],
                                        in1=sg[:, 3 * HB:4 * HB], op=ALU.mult)
                nc.vector.tensor_tensor(out=cs, in0=t1[:], in1=t2[:], op=ALU.add)
                sgs.append(sg)
            for n in range(NBLK):
                # phase B: h production + transpose back for the next step
                sg = sgs[n]
                cs = c_st[:, HB * n:HB * (n + 1)]
                tch = wkpool.tile([Bl, HB], BF16, tag=f"tch{n}", name="tch")
                nc.scalar.activation(tch[:], cs, AF.Tanh)
                nc.vector.tensor_tensor(
                    out=ring[0][:, t % RC, HB * n:HB * (n + 1)],
                    in0=sg[:, 2 * HB:3 * HB], in1=tch[:], op=ALU.mult)
                hT_ps = tppool.tile([128, CPB * Bl], F32, tag="hTp", name="hT_ps")
                for kk in range(CPB):
                    lo = HB * n + 128 * kk
                    nc.tensor.matmul(
                        hT_ps[:, Bl * kk:Bl * (kk + 1)],
                        ring[0][:, t % RC, lo:lo + 128], id_sb[:],
                        start=True, stop=True)
                hTn = htpool.tile([128, CPB * Bl], BF16, tag=f"hT{n}", name="hTn")
                nc.vector.tensor_copy(hTn[:], hT_ps[:])
                hT[n] = hTn
            if (t + 1) % RC == 0:
                t0r = t + 1 - RC
                nc.gpsimd.dma_start(houtv[:, t0r:t0r + RC, :], ring[0][:])
                if coll is not None:
                    # stream the f/b hidden-state exchange under the scan:
                    # once 64 more steps (2048 t-major rows) are flushed,
                    # AllGather just that row range; the final chunk also
                    # carries the zero pad rows
                    h1a, groups = coll
                    NPr = hout.shape[0]
                    CSTEP = 64 * Bl
                    outv = h1a.ap().rearrange("(g r) d -> g r d", g=2)
                    j = (t + 1) // 64 - 1
                    if (t + 1) % 64 == 0 and j < 7:
                        r0, r1 = j * CSTEP, (j + 1) * CSTEP
                        nc.gpsimd.collective_compute(
                            "AllGather", ALU.bypass, replica_groups=groups,
                            ins=[hout.ap()[r0:r1, :].opt()],
                            outs=[outv[:, r0:r1, :].opt()])
                    elif t + 1 == Tn:
                        r0 = 7 * CSTEP
                        nc.gpsimd.collective_compute(
                            "AllGather", ALU.bypass, replica_groups=groups,
                            ins=[hout.ap()[r0:NPr, :].opt()],
                            outs=[outv[:, r0:NPr, :].opt()])
                if cls is not None:
                    wcls_sb, logitsT = cls
                    c = t0r // RC
                    o2 = clgp.tile([128, H], BF16, tag="clin")
                    nc.scalar.dma_start(o2[:], hout[128 * c:128 * (c + 1), :])
                    o2T = clgtp.tile([128, KH, 128], BF16, tag="clinT")
                    for k in range(KH):
                        nc.sync.dma_start_transpose(
                            o2T[:, k, :], o2[:, 128 * k:128 * (k + 1)])
                    cps = clpp.tile([TAGS, 128], F32, name="cps_t")
                    for k in range(KH):
                        nc.tensor.matmul(cps[:], wcls_sb[:, k, :], o2T[:, k, :],
                                         start=(k == 0), stop=(k == KH - 1))
                    lg = clop.tile([TAGS, 128], F32, tag="cllg")
                    nc.scalar.activation(lg[:], cps[:], AF.Copy)
                    nc.scalar.dma_start(logitsT[:, 128 * c:128 * (c + 1)], lg[:])


def _prep_inputs(inputs, Tn=T, Bl=BL, ncores=NC):
    x = np.asarray(inputs["x"]).astype(np.int32)
    lengths = np.asarray(inputs["lengths"]).astype(np.int32)
    emb = np.asarray(inputs["emb"], dtype=np.float32)
    ntok = Bl * Tn
    NP = ntok + 128
    ZF = ntok          # zero row in local / f-half of h1_all
    ZB = NP + ntok     # zero row in b-half of h1_all

    com = {"ident": np.eye(Bl, dtype=ml_dtypes.bfloat16)}

    def prep_dir(fwd):
        d = {}
        for lyr, (si, sh, sb) in {1: ("W_ih_f1", "W_hh_f1", "b_f1") if fwd else
                                     ("W_ih_b1", "W_hh_b1", "b_b1"),
                                  2: ("W_ih_f2", "W_hh_f2", "b_f2") if fwd else
                                     ("W_ih_b2", "W_hh_b2", "b_b2")}.items():
            w_ih = np.asarray(inputs[si], np.float32)[_GPERM]
            w_hh = np.asarray(inputs[sh], np.float32)[_GPERM]
            bb = np.asarray(inputs[sb], np.float32)[_GPERM]
            d[f"wihT_{lyr}"] = np.ascontiguousarray(w_ih.T).astype(ml_dtypes.bfloat16)
            d[f"whhT_{lyr}"] = np.ascontiguousarray(w_hh.T).astype(ml_dtypes.bfloat16)
            d[f"bias_{lyr}"] = np.tile(bb.reshape(1, G), (128, 1)).astype(np.float32)
        wc = np.asarray(inputs["W_cls"], np.float32)  # [TAGS, 2H]
        half = wc[:, :H] if fwd else wc[:, H:]
        d["wclsT"] = np.ascontiguousarray(half.T).astype(ml_dtypes.bfloat16)
        return d

    dir_maps = {True: prep_dir(True), False: prep_dir(False)}

    def chunked(a2d):  # [Bl, Tn] -> t-major flat -> [128, ntok//128]
        a = np.ascontiguousarray(a2d.T).reshape(-1)
        return np.ascontiguousarray(a.reshape(ntok // 128, 128).T)

    in_maps = []
    for c in range(ncores):
        g = c % 4
        fwd = c < 4
        xs = x[Bl * g:Bl * (g + 1), :Tn]
        ls = np.minimum(lengths[Bl * g:Bl * (g + 1)], Tn)
        ts = np.arange(Tn)[None, :]
        valid = ts < ls[:, None]
        rev = np.where(valid, ls[:, None] - 1 - ts, ts)    # [Bl,Tn]

        bcol = np.arange(Bl)[:, None]  # t-major: row(b, t) = t*Bl + b
        if fwd:
            x_ids = xs                                 # scan order = natural
            # proj2 token (b,t): f-part row (b,t), b-part row NP + (b, rev t)
            p2f = np.where(valid, ts * Bl + bcol, ZF)
            p2b = np.where(valid, NP + rev * Bl + bcol, ZB)
        else:
            x_ids = np.take_along_axis(xs, rev, axis=1)  # scan order = reversed
            # scan position s corresponds to original t = len-1-s (s<len).
            # input token at s: f-part row (b, len-1-s) = (b, rev s) in f half,
            # b-part row (b, s) in b half.
            p2f = np.where(valid, rev * Bl + bcol, ZF)
            p2b = np.where(valid, NP + ts * Bl + bcol, ZB)

        # host-side embedding gather, t-major scan order, bf16
        e_rows = emb[np.ascontiguousarray(x_ids.T).reshape(-1)]
        m = {
            "e_rows": np.ascontiguousarray(e_rows).astype(ml_dtypes.bfloat16),
            "p2f_idx": chunked(p2f.astype(np.int32)),
            "p2b_idx": chunked(p2b.astype(np.int32)),
        }
        m.update(com)
        m.update(dir_maps[fwd])
        in_maps.append(m)
    return in_maps


_CACHED = {}


def kernel(**inputs) -> np.ndarray:
    if "nc" not in _CACHED:
        nc = bacc.Bacc("TRN2", target_bir_lowering=False, debug=False,
                       num_devices=NC)
        _build(nc)
        nc.compile()
        _CACHED["nc"] = nc
    nc = _CACHED["nc"]
    in_maps = _prep_inputs(inputs)
    res = run_bass_kernel_spmd(nc, in_maps, core_ids=list(range(NC)), trace=False)
    lengths = np.minimum(np.asarray(inputs["lengths"]).astype(np.int64), T)
    b_cls = np.asarray(inputs["b_cls"], np.float32)
    ts = np.arange(T)[None, :]
    outs = []
    for g in range(4):
        ls = lengths[BL * g:BL * (g + 1)]
        valid = (ts < ls[:, None])[:, :, None]
        rev = np.where(ts < ls[:, None], ls[:, None] - 1 - ts, 0)
        lf = res.results[g]["logitsT"].astype(np.float32)
        lb = res.results[g + 4]["logitsT"].astype(np.float32)
        Lf = np.ascontiguousarray(lf.T).reshape(T, BL, TAGS).transpose(1, 0, 2)
        Lb = np.ascontiguousarray(lb.T).reshape(T, BL, TAGS).transpose(1, 0, 2)
        Lb_nat = np.take_along_axis(Lb, rev[:, :, None], axis=1)
        outs.append(np.where(valid, Lf + Lb_nat, 0.0) + b_cls)
    return np.concatenate(outs, axis=0).astype(np.float32)
